# revision 26
# baseline (speedup 1.0000x reference)
"""Causal self-attention (B=1, T=2048, E=2048, 16 heads, RoPE) on 8 TRN2 NeuronCores.

Strategy: tensor-parallel over heads (2 heads/core). Each core computes
QKV for its heads, RoPE, causal softmax attention, and a PARTIAL output
projection over its 256 contraction columns of w_proj. The host sums the
8 partial [T, E] outputs (no on-device collectives).

v3 performance changes vs v2 (215-242us):
  - PV + rowsum fused in ONE matmul stream: P^T 128x128 tiles are the
    STATIONARY operand, the moving operand is [v_h | ones] (129 cols), so
    the softmax denominator drops out as column 128/0 of the PV psum.
    Replaces 160 matmuls x (512 moving + 128 ldweights) with 272 x
    (129 + 128): ~24k PE cycles saved, and the separate ss psum bank,
    the DVE [1,512] reciprocal and the GPSIMD partition_broadcast chain
    all disappear (normalize is a native per-partition tensor_scalar).
  - attention comes out [q, d]; one PE transpose per 128x128 tile
    (bf16, via identity) restores [d, q] for the projection stationary.
  - q/k stored bf16 (not f32r): removes the 4x f32r penalty on score
    matmuls with moving < 256 cols and halves qk SBUF.
  - xT DRAM layout [j, q, p, c, b]: every x DMA is a contiguous-row 2D
    pattern (hardware ring, not software-descriptor), and chunk-0 pieces
    + weight tiles are interleaved across the 3 queues in consumption
    order -> the PE's first QKV never starves (was ~19us of startup idle).
  - projection emits per 128-row tile right after head-1's transpose of
    that tile (lagged by one to hide the recip/mul/transpose/copy chain),
    so output DMAs spread across the run and the tail shrinks to the
    last tile's proj+DMA only.
  - band-tile zero-prefix memsets dropped: flipped PV bursts only ever
    read column blocks >= the tile's diagonal offset.
"""
import sys

for _p in ("/opt/trn_rl_repo",):
    if _p not in sys.path:
        sys.path.append(_p)

import numpy as np

B, T, E = 1, 2048, 2048
H, D = 16, 128
N_CORES = 8
HPC = H // N_CORES          # heads per core
CL = HPC * D                # contraction columns per core (256)
QC = 512                    # qt chunk (PSUM bank width in fp32)
BASE = 10000.0

_CACHE: dict = {}


# ---------------------------------------------------------------- device build
def _build_nc(t=T, debug_stop=None):
    import concourse.tile as tile
    from concourse import bacc, mybir
    from contextlib import ExitStack

    f32 = mybir.dt.float32
    bf16 = mybir.dt.bfloat16
    nj = t // QC            # qt chunks
    ntt = t // 128          # t tiles
    nct = E // 128          # contraction tiles
    nhalf = nct // 2

    nc = bacc.Bacc("TRN2", target_bir_lowering=False, debug=False,
                   enable_asserts=False, num_devices=N_CORES)
    # x^T, partition-major: [chunk, half, p(128), c(8), b(512)] so every
    # DMA row (fixed p) is contiguous in DRAM
    xT_d = nc.dram_tensor("xT", [t // QC, 2, 128, nhalf, QC], bf16,
                          kind="ExternalInput").ap()
    wqkvT_d = nc.dram_tensor("wqkvT", [E, 6 * 128], bf16, kind="ExternalInput").ap()
    wprojT_d = nc.dram_tensor("wprojT", [CL, E], bf16, kind="ExternalInput").ap()
    cos2_d = nc.dram_tensor("cos2", [128, t], f32, kind="ExternalInput").ap()
    sin2_d = nc.dram_tensor("sin2", [128, t], f32, kind="ExternalInput").ap()
    tri_d = nc.dram_tensor("trimask", [128, 128], bf16, kind="ExternalInput").ap()
    ident_d = nc.dram_tensor("ident", [128, 128], bf16, kind="ExternalInput").ap()
    out_d = nc.dram_tensor("out", [t, E], bf16, kind="ExternalOutput").ap()

    Exp = mybir.ActivationFunctionType.Exp

    with tile.TileContext(nc) as tc:
        with ExitStack() as per:  # persistent pools
            wpp = per.enter_context(tc.tile_pool(name="wpp", bufs=1))
            qkp = per.enter_context(tc.tile_pool(name="qkp", bufs=1))
            vp = per.enter_context(tc.tile_pool(name="vp", bufs=1))
            atp = per.enter_context(tc.tile_pool(name="atp", bufs=1))
            ps = per.enter_context(tc.tile_pool(name="ps", bufs=1, space="PSUM"))

            # persistent activations
            # qk: 4 f-groups (q0 q1 k0 k1), bf16
            qk_sb = [qkp.tile([128, t], bf16, tag=f"qk{f}", name=f"qk{f}")
                     for f in range(4)]
            # v: per 128-t-tile [v_h0 (0:128) | ones (128) | v_h1 (129:257)]
            v_sb = [vp.tile([128, 264], bf16, tag=f"v{g}", name=f"v{g}")
                    for g in range(ntt)]
            # attention output, transposed back to [d, t], per head
            attnT_sb = [atp.tile([128, t], bf16, tag=f"at{h}", name=f"at{h}")
                        for h in range(HPC)]
            for g in range(ntt):
                nc.vector.memset(v_sb[g][:, 128:129], 1.0)

            wqp = per.enter_context(tc.tile_pool(name="wqp", bufs=1))
            xtr = per.enter_context(tc.tile_pool(name="xtr", bufs=4))
            rtmp = per.enter_context(tc.tile_pool(name="rtmp", bufs=3))
            cstp = per.enter_context(tc.tile_pool(name="cstp", bufs=1))
            ptp = per.enter_context(tc.tile_pool(name="ptp", bufs=36))
            mkp = per.enter_context(tc.tile_pool(name="mkp", bufs=1))
            outp = per.enter_context(tc.tile_pool(name="outp", bufs=3))
            stg = per.enter_context(tc.tile_pool(name="stg", bufs=4))

            cos2_sb = cstp.tile([128, t], f32)
            sin2_sb = cstp.tile([128, t], f32)
            tri_sb = mkp.tile([128, 128], bf16)
            ident_sb = mkp.tile([128, 128], bf16)

            # ---------------- startup DMA choreography ----------------
            # 3 queues (sync, scalar, gpsimd). Chunk-0 x pieces + weight
            # tiles interleaved in PE consumption order (b_pq(0) is
            # c-outer): per c-pair, sync gets x half-0, scalar x half-1,
            # gpsimd the two weight tiles. Masks ride gpsimd first
            # (tiny); cos/sin chunk-0 slices ride after the 3rd pair.
            wq_sb = [wqp.tile([128, 6 * 128], bf16, tag=f"w{c}", name=f"w{c}")
                     for c in range(nct)]
            xts_map = {}

            def emit_x0():
                # chunk-0: c0/c8 land as singles so the PE's first matmul
                # starts ~2us earlier; weights spread over all 3 queues
                # (gpsimd w0-6, sync w7-11 after its x half, scalar
                # w12-15) so no queue gates the c-outer QKV stream
                xh0 = xtr.tile([128, nhalf * QC], bf16, tag="xt", name="xt0_0")
                xh1 = xtr.tile([128, nhalf * QC], bf16, tag="xt", name="xt0_1")

                def w_dma(eng, c):
                    # qk columns only: the startup-critical 2/3 of the tile
                    eng.dma_start(out=wq_sb[c][:, 0:512],
                                  in_=wqkvT_d[c * 128:(c + 1) * 128, 0:512])

                def wv_dma(eng, c):
                    # v columns: first needed by b_pv(0), well after QKV
                    eng.dma_start(out=wq_sb[c][:, 512:768],
                                  in_=wqkvT_d[c * 128:(c + 1) * 128, 512:768])

                def x_dma(eng, xh, q, c0, c1):
                    eng.dma_start(
                        out=xh[:, c0 * QC:c1 * QC].rearrange(
                            "p (c b) -> p c b", c=c1 - c0),
                        in_=xT_d[0, q, :, c0:c1])
                w_dma(nc.gpsimd, 0)
                x_dma(nc.sync, xh0, 0, 0, 1)
                x_dma(nc.scalar, xh1, 1, 0, 1)
                w_dma(nc.gpsimd, 1)
                w_dma(nc.gpsimd, 2)
                for i in range(3):
                    x_dma(nc.sync, xh0, 0, 2 * i + 1, 2 * i + 3)
                    x_dma(nc.scalar, xh1, 1, 2 * i + 1, 2 * i + 3)
                    w_dma(nc.gpsimd, 3 + i)
                    if i == 0:
                        nc.gpsimd.dma_start(out=tri_sb[:], in_=tri_d[:])
                        nc.gpsimd.dma_start(out=ident_sb[:], in_=ident_d[:])
                x_dma(nc.sync, xh0, 0, 7, 8)
                x_dma(nc.scalar, xh1, 1, 7, 8)
                w_dma(nc.gpsimd, 6)
                for c in range(7, 12):
                    w_dma(nc.sync, c)
                for c in range(12, 16):
                    w_dma(nc.scalar, c)
                # rope tables + deferred v-columns AFTER the qk weight
                # tails: needed only once the c-outer QKV block has drained
                jsl = slice(0, QC)
                nc.sync.dma_start(out=cos2_sb[:, jsl], in_=cos2_d[:, jsl])
                nc.scalar.dma_start(out=sin2_sb[:, jsl], in_=sin2_d[:, jsl])
                for c in range(7):
                    wv_dma(nc.gpsimd, c)
                for c in range(7, 12):
                    wv_dma(nc.sync, c)
                for c in range(12, 16):
                    wv_dma(nc.scalar, c)
                xts = []
                for xh in (xh0, xh1):
                    for cc in range(nhalf):
                        xts.append(xh[:, cc * QC:(cc + 1) * QC])
                xts_map[0] = xts

            def emit_x(j):
                jsl = slice(j * QC, (j + 1) * QC)
                xts = []
                for q in range(2):
                    xh = xtr.tile([128, nhalf * QC], bf16, tag="xt",
                                  name=f"xt{j}_{q}")
                    eng = nc.sync if q == 0 else nc.scalar
                    eng.dma_start(
                        out=xh[:].rearrange("p (c b) -> p c b", c=nhalf),
                        in_=xT_d[j, q])
                    for cc in range(nhalf):
                        xts.append(xh[:, cc * QC:(cc + 1) * QC])
                xts_map[j] = xts
                nc.sync.dma_start(out=cos2_sb[:, jsl], in_=cos2_d[:, jsl])
                nc.scalar.dma_start(out=sin2_sb[:, jsl], in_=sin2_d[:, jsl])

            wp_sb = []

            def emit_wp_dmas():
                for hh in range(HPC):
                    w = wpp.tile([128, E], bf16, tag=f"wp{hh}", name=f"wp{hh}")
                    nc.gpsimd.dma_start(out=w[:],
                                        in_=wprojT_d[hh * 128:(hh + 1) * 128, :])
                    wp_sb.append(w)

            def b_pq(j):
                # j=0: contraction-outer so each weight tile feeds 4
                # matmuls as it lands. j>0: f-outer (needs only ONE free
                # bank to start and lets RoPE begin 3 f-groups earlier).
                jsl = slice(j * QC, (j + 1) * QC)
                xts = xts_map[j]
                pqs = [ps.tile([128, QC], f32, tag="a", bufs=4,
                               name=f"pq{j}_{f}") for f in range(4)]
                if j == 0:
                    for c in range(nct):
                        for f in range(4):
                            nc.tensor.matmul(
                                pqs[f][:], wq_sb[c][:, f * 128:(f + 1) * 128],
                                xts[c], start=(c == 0), stop=(c == nct - 1))
                else:
                    for f in range(4):
                        for c in range(nct):
                            nc.tensor.matmul(
                                pqs[f][:], wq_sb[c][:, f * 128:(f + 1) * 128],
                                xts[c], start=(c == 0), stop=(c == nct - 1))
                for f in range(4):
                    pq = pqs[f]
                    # ACT stages the half-SWAPPED pq into SBUF (psum->sbuf
                    # crosses spaces, so the partition-base shift is legal)
                    # -> the pq psum slot frees after tA + these two quick
                    # copies instead of the full serial DVE chain, and tB
                    # becomes ONE full-height mul (3 DVE ops, was 4)
                    sgw = rtmp.tile([128, QC], f32, tag="pqs", bufs=2,
                                    name=f"sgw{j}_{f}")
                    nc.scalar.copy(sgw[0:64, :], pq[64:128, :])
                    nc.scalar.copy(sgw[64:128, :], pq[0:64, :])
                    # RoPE: out = pq*cos2 + swap_halves(pq)*sin2, sin2=[-sin; sin]
                    tA = rtmp.tile([128, QC], f32, tag="tA", name=f"tA{j}_{f}")
                    nc.vector.tensor_mul(tA[:], pq[:], cos2_sb[:, jsl])
                    tB = rtmp.tile([128, QC], f32, tag="tB", name=f"tB{j}_{f}")
                    nc.vector.tensor_mul(tB[:], sgw[:], sin2_sb[:, jsl])
                    nc.vector.tensor_add(qk_sb[f][:, jsl], tA[:], tB[:])

            def b_pv(j):
                # v natural [t, d] per 128-t-tile; psum [128, 256] copied
                # into the [v_h0 | ones | v_h1] layout in two halves
                # (ACT + Pool, keeping DVE free for RoPE)
                xts = xts_map.pop(j)
                for tt in range(4):
                    pv = ps.tile([128, 2 * D], f32, tag="c", bufs=3,
                                 name=f"pv{j}_{tt}")
                    for c in range(nct):
                        nc.tensor.matmul(pv[:], xts[c][:, tt * 128:(tt + 1) * 128],
                                         wq_sb[c][:, 4 * 128:6 * 128],
                                         start=(c == 0), stop=(c == nct - 1))
                    g = j * 4 + tt
                    nc.scalar.copy(v_sb[g][:, 0:128], pv[:, 0:128])
                    nc.vector.tensor_copy(v_sb[g][:, 129:257], pv[:, 128:256])

            def c_scores(j, h):
                nkt = 4 * (j + 1)
                pts = {}
                for k in range(nkt):
                    o = k - 4 * j
                    stp = ps.tile([128, QC], f32, tag="a", bufs=4,
                                  name=f"st{j}_{h}_{k}")
                    # band tiles: only columns >= o*128 are valid
                    mc0 = o * 128 if o > 0 else 0
                    nc.tensor.matmul(
                        stp[:, mc0:QC],
                        qk_sb[2 + h][:, k * 128:(k + 1) * 128],
                        qk_sb[h][:, j * QC + mc0:(j + 1) * QC],
                        start=True, stop=True)
                    pt = ptp.tile([128, QC], bf16, tag="pt",
                                  name=f"pt{j}_{h}_{k}")
                    if o < 0:
                        nc.scalar.activation(pt[:], stp[:], Exp)
                    else:
                        # ONE exp over the whole valid range, then the
                        # diagonal 128-col block in-place through the
                        # shared triangle mask on the Pool engine. The
                        # zero prefix is never read by the flipped PV
                        # bursts, so no memset needed.
                        c0 = o * 128
                        nc.scalar.activation(pt[:, c0:QC], stp[:, c0:QC], Exp)
                        nc.gpsimd.tensor_mul(pt[:, c0:c0 + 128],
                                             pt[:, c0:c0 + 128], tri_sb[:])
                    pts[k] = pt
                return pts

            # lagged PE work: transposes and proj tiles are emitted one
            # burst late so their DVE dependency chains are hidden
            pe_backlog = []

            def drain_backlog(nmax=None):
                n = len(pe_backlog) if nmax is None else min(nmax, len(pe_backlog))
                for _ in range(n):
                    pe_backlog.pop(0)()

            def emit_transpose(j, h, qq, po2, inv):
                tt = 4 * j + qq
                zc, a0 = (128, 0) if h == 0 else (0, 1)
                nc.vector.reciprocal_approx_fast(out=inv[:],
                                                 in_=po2[:, zc:zc + 1])
                astg = stg.tile([128, 128], bf16, tag="as",
                                name=f"as{j}_{h}_{qq}")
                nc.vector.tensor_scalar_mul(astg[:], po2[:, a0:a0 + 128],
                                            inv[:])

                def tr(tt=tt, h=h, astg=astg):
                    tp = ps.tile([128, 128], bf16, tag="t", bufs=1,
                                 name=f"tp{j}_{h}_{qq}")
                    nc.tensor.transpose(tp[:], astg[:], ident_sb[:])
                    eng = nc.scalar if (tt + h) % 2 == 0 else nc.vector
                    if eng is nc.scalar:
                        eng.copy(attnT_sb[h][:, tt * 128:(tt + 1) * 128], tp[:])
                    else:
                        eng.tensor_copy(attnT_sb[h][:, tt * 128:(tt + 1) * 128],
                                        tp[:])
                pe_backlog.append(tr)

            def emit_proj(tt, last=False):
                # projection + output DMA for one 128-row tile
                def pj(tt=tt):
                    ob = outp.tile([128, E], bf16, tag="ob", name=f"ob{tt}")
                    for oc in range(E // 512):
                        pp = ps.tile([128, 512], f32, tag="a", bufs=4,
                                     name=f"pp{tt}_{oc}")
                        for h in range(HPC):
                            nc.tensor.matmul(
                                pp[:], attnT_sb[h][:, tt * 128:(tt + 1) * 128],
                                wp_sb[h][:, oc * 512:(oc + 1) * 512],
                                start=(h == 0), stop=(h == HPC - 1))
                        osl = slice(oc * 512, (oc + 1) * 512)
                        if oc % 2 == 0:
                            nc.vector.tensor_copy(ob[:, osl], pp[:])
                        else:
                            nc.scalar.copy(ob[:, osl], pp[:])
                        if last:
                            # final tile: stream out per-oc across THREE
                            # queues (scalar is idle by now) so the tail
                            # DMA drain is one 128KB piece per queue
                            eng = (nc.sync, nc.gpsimd, nc.scalar,
                                   nc.sync)[oc]
                            eng.dma_start(
                                out=out_d[tt * 128:(tt + 1) * 128, osl],
                                in_=ob[:, osl])
                    if not last:
                        eng = nc.sync if tt % 2 == 0 else nc.gpsimd
                        eng.dma_start(out=out_d[tt * 128:(tt + 1) * 128, :],
                                      in_=ob[:])
                pe_backlog.append(pj)

            def c_pv(j, h, pts):
                # flipped PV: stationary = P^T [k,128 q] tiles, moving =
                # [v_h | ones] (129 cols). po2 col 128 (h0) / col 0 (h1)
                # is the softmax denominator.
                vbase = 0 if h == 0 else 128
                for qq in range(4):
                    nk = 4 * j + qq + 1
                    po2 = ps.tile([128, 132], f32, tag="c", bufs=3,
                                  name=f"po{j}_{h}_{qq}")
                    for k in range(nk):
                        nc.tensor.matmul(
                            po2[:, 0:129],
                            pts[k][:, qq * 128:(qq + 1) * 128],
                            v_sb[k][:, vbase:vbase + 129],
                            start=(k == 0), stop=(k == nk - 1))
                    drain_backlog(1 if h == 0 else 2)
                    inv = stg.tile([128, 1], f32, tag="inv",
                                   name=f"inv{j}_{h}_{qq}")
                    emit_transpose(j, h, qq, po2, inv)
                    if h == 1:
                        emit_proj(4 * j + qq, last=(4 * j + qq == ntt - 1))

            # ---------------------------- main schedule ----------------------------
            emit_x0()
            b_pq(0)
            b_pv(0)
            emit_x(1)
            emit_wp_dmas()
            for j in range(nj):
                pts0 = c_scores(j, 0)
                pts1 = c_scores(j, 1)
                if j < nj - 1:
                    b_pq(j + 1)
                c_pv(j, 0, pts0)
                c_pv(j, 1, pts1)
                if j < nj - 1:
                    b_pv(j + 1)
                    if j + 2 < nj:
                        emit_x(j + 2)
            drain_backlog()

            if debug_stop == "C":
                for h in range(HPC):
                    ob = outp.tile([128, t], bf16, tag="obC", name=f"obC{h}")
                    nc.vector.tensor_copy(ob[:], attnT_sb[h][:])
                    nc.sync.dma_start(out=out_d[h * 128:(h + 1) * 128, 0:t],
                                      in_=ob[:])

    nc.compile()
    return nc


# ---------------------------------------------------------------- host prep
def _rope_perm():
    p = np.empty(E, dtype=np.int64)
    for h in range(H):
        b = h * D
        p[b:b + 64] = b + np.arange(0, D, 2)
        p[b + 64:b + D] = b + np.arange(1, D, 2)
    return p


def _tables(t=T):
    # match reference: fp32 theta, fp32 angles
    theta = (1.0 / (BASE ** (np.arange(0, D, 2, dtype=np.float32) / np.float32(D)))
             ).astype(np.float32)
    m = np.arange(t, dtype=np.float32)
    fr = np.outer(m, theta).astype(np.float32)        # [t, 64]
    cos = np.cos(fr).T.astype(np.float32)             # [64, t]
    sin = np.sin(fr).T.astype(np.float32)
    cos2 = np.ascontiguousarray(np.concatenate([cos, cos], 0))
    sin2 = np.ascontiguousarray(np.concatenate([-sin, sin], 0))
    return cos2, sin2


def _trimask():
    import ml_dtypes
    a = np.arange(128)[:, None]
    b = np.arange(128)[None, :]
    return np.ascontiguousarray((b >= a).astype(ml_dtypes.bfloat16))


def _ident():
    import ml_dtypes
    return np.ascontiguousarray(np.eye(128, dtype=ml_dtypes.bfloat16))


def _prep_inputs(x, w_attn, w_proj, t=T):
    import ml_dtypes
    bf16 = ml_dtypes.bfloat16
    x2 = np.asarray(x, dtype=np.float32).reshape(t, E)
    # [j, q, p(128), c(8), b(512)]: partition-major, contiguous rows
    xT = np.ascontiguousarray(
        x2.T.reshape(2, E // 256, 128, t // QC, QC).transpose(3, 0, 2, 1, 4)
    ).astype(bf16)
    perm = _rope_perm()
    scale = np.float32(1.0) / np.sqrt(np.float32(D))
    wq = np.asarray(w_attn[0:E])[perm] * scale
    wk = np.asarray(w_attn[E:2 * E])[perm]
    wv = np.asarray(w_attn[2 * E:3 * E])
    cos2, sin2 = _tables(t)
    tri = _trimask()
    ident = _ident()
    in_maps = []
    for c in range(N_CORES):
        rows = slice(c * CL, (c + 1) * CL)
        wqkv = np.concatenate([wq[rows], wk[rows], wv[rows]], axis=0)  # [768, E]
        in_maps.append({
            "xT": xT,
            "wqkvT": np.ascontiguousarray(wqkv.T).astype(bf16),
            "wprojT": np.ascontiguousarray(np.asarray(w_proj)[:, rows].T
                                           ).astype(bf16),
            "cos2": cos2,
            "sin2": sin2,
            "trimask": tri,
            "ident": ident,
        })
    return in_maps


# ---------------------------------------------------------------- cached runner
def _get_runner(t=T, debug_stop=None):
    """Build the Bass module once and return a cached jitted executor.

    Mirrors concourse.bass2jax.run_bass_via_pjrt's multi-core branch, but
    keeps the jitted callable so repeated kernel() calls don't recompile.
    """
    key = ("runner", t, debug_stop)
    if key in _CACHE:
        return _CACHE[key]
    import jax
    from concourse import bass2jax, mybir
    from jax.experimental.shard_map import shard_map
    from jax.sharding import Mesh, PartitionSpec

    nc = _build_nc(t, debug_stop)
    bass2jax.install_neuronx_cc_hook()

    partition_name = (nc.partition_id_tensor.name if nc.partition_id_tensor
                      else None)
    in_names, out_names, out_avals, zero_shapes = [], [], [], []
    for alloc in nc.m.functions[0].allocations:
        if not isinstance(alloc, mybir.MemoryLocationSet):
            continue
        name = alloc.memorylocations[0].name
        if alloc.kind == "ExternalInput":
            if name != partition_name:
                in_names.append(name)
        elif alloc.kind == "ExternalOutput":
            shape = tuple(alloc.tensor_shape)
            dtype = mybir.dt.np(alloc.dtype)
            out_names.append(name)
            out_avals.append(jax.core.ShapedArray(shape, dtype))
            zero_shapes.append((shape, dtype))
    n_params = len(in_names)
    all_in_names = list(in_names) + list(out_names)
    if partition_name is not None:
        all_in_names.append(partition_name)

    def _body(*args):
        operands = list(args)
        if partition_name is not None:
            operands.append(bass2jax.partition_id_tensor())
        outs = bass2jax._bass_exec_p.bind(
            *operands,
            out_avals=tuple(out_avals),
            in_names=tuple(all_in_names),
            out_names=tuple(out_names),
            lowering_input_output_aliases=(),
            sim_require_finite=True,
            sim_require_nnan=True,
            nc=nc,
        )
        return tuple(outs)

    devices = jax.devices()[:N_CORES]
    mesh = Mesh(np.asarray(devices), ("core",))
    donate = tuple(range(n_params, n_params + len(out_names)))
    sharded = jax.jit(
        shard_map(_body, mesh=mesh,
                  in_specs=(PartitionSpec("core"),) * (n_params + len(out_names)),
                  out_specs=(PartitionSpec("core"),) * len(out_names)),
        donate_argnums=donate, keep_unused=True)

    runner = {"fn": sharded, "in_names": in_names, "out_names": out_names,
              "out_avals": out_avals, "zero_shapes": zero_shapes, "nc": nc}
    _CACHE[key] = runner
    return runner


def _run(in_maps, t=T, debug_stop=None):
    r = _get_runner(t, debug_stop)
    concat_in = [
        np.concatenate([np.asarray(in_maps[c][name]) for c in range(N_CORES)],
                       axis=0)
        for name in r["in_names"]
    ]
    concat_zeros = [np.zeros((N_CORES * s[0], *s[1:]), d)
                    for (s, d) in r["zero_shapes"]]
    out_arrs = r["fn"](*concat_in, *concat_zeros)
    outs = []
    for c in range(N_CORES):
        outs.append({
            name: np.asarray(out_arrs[i]).reshape(N_CORES,
                                                  *r["out_avals"][i].shape)[c]
            for i, name in enumerate(r["out_names"])
        })
    return outs


# ---------------------------------------------------------------- entry point
def kernel(x, w_attn, w_proj):
    x = np.asarray(x, dtype=np.float32)
    w_attn = np.asarray(w_attn, dtype=np.float32)
    w_proj = np.asarray(w_proj, dtype=np.float32)
    in_maps = _prep_inputs(x, w_attn, w_proj)
    outs = _run(in_maps)
    acc = outs[0]["out"].astype(np.float32)
    for c in range(1, N_CORES):
        acc = acc + outs[c]["out"].astype(np.float32)
    return acc.reshape(B, T, E).astype(np.float32)
